# revision 4
# baseline (speedup 1.0000x reference)
"""Biquad peaking-EQ IIR on 8 Trainium2 NeuronCores — fp8 residual edition.

Math: the reference filter y = H(x) is a 2nd-order IIR whose impulse response
decays below 1e-10 after 256 taps; we compute the *residual* d = y - x as a
truncated-FIR convolution (taps g = h - delta), quantize everything to
fp8 e4m3, and reconstruct y = x + d on the host (which already holds x in
fp32).  End-to-end rel-L2 error ~6e-3 (numpy-validated), under the 2e-2 gate.

Performance design (59.7us bf16 ancestor -> ~45.5us measured):
 1. fp8 e4m3 input AND output halve DMA bytes to 8.4 MB/core -> ~24us of
    HBM flow at the ~350 B/ns per-core aggregate rate.
 2. Two plain fp8 matmuls per 512-block chunk (T0 x X[c] + T1 x X[c-1],
    fp32 PSUM accumulation); back-to-back MMs issue every ~216 ns (full
    2.4 GHz), so the 128-MM PE stream is ~27.6us and is the critical
    stream.  (fp8 DoubleRow would halve MM count but requires the moving
    k-pair stride to be a multiple of 16 bytes: overlapping stride-1
    views crash the PE, a 16-byte-interleaved layout runs the strided
    fetch 2.8x slow, and building an on-chip shifted duplicate costs more
    than it saves - SBUF->SBUF DMA measured ~163-260 B/ns with multi-us
    completion lag, and engine copies are parity-blocked on the odd
    1-byte shift.  All measured; plain wins.)
 3. Work streams in 4 column-slabs (512 block cols x 16 half-signal
    units, ~1.05 MB).  Each slab tile carries its own leading halo col
    per unit (host-duplicated), so slabs are fully independent; loads and
    stores split across the SP and GpSimd HWDGE rings (2 queues x ~200
    B/ns covers the ~350 B/ns core cap); compute is slab-major so output
    retires evenly; stores go out in unit-group quarters (eighths on the
    last slab) right behind the evacuations to minimize the tail.
 4. PSUM chunk tiles [128,512] f32 (7 bufs); evacuation (fp32->fp8 cast,
    ~687ns/chunk) alternates DVE/ACT per unit (~22us each, hidden under
    the PE stream).  The Toeplitz pair rides the front of slab 0's first
    load piece; slab 0 loads in small pieces so the first MM fires ~10.5us.
 5. PE pstate warmup: 5 dummy matmuls on a zeroed scratch tile keep the
    PE busy from ~8.3us so real MMs reach the full clock quickly (the
    PE downclocks unless continuously busy ~3us).

Scheduling note: TPB instructions have a single semaphore-wait slot; Tile's
slot-release deps routinely emit 2+ waits.  _strip_redundant_waits
post-processes the scheduled BIR (see its docstring).
"""

import math

import numpy as np

SAMPLE_RATE = 44100.0

# Problem geometry (hardcoded per harness contract).
B_FULL, C_FULL, T_FULL = 32, 2, 524288
N_CORES = 8
SIGS = B_FULL * C_FULL          # 64 signals
SPC = SIGS // N_CORES           # 8 signals per core
L = 128                         # block size == PE array dim
NBLK = T_FULL // L              # 4096 block cols per signal
NU = 16                         # interleaved half-signal units per core
UW = NBLK // 2                  # 2048 block cols per unit
QW = 512                        # block cols per chunk (= PSUM bank, fp32)
NSLAB = UW // QW                # 4 slabs
SROW = NU * (1 + QW)            # slab row bytes: 16 units x (halo col + 512)
WBYTES = 2 * L                  # Toeplitz pair rides the front of slab 0


def _filter_coeffs(center_freq: float, q: float, gain: float):
    """torchaudio equalizer_biquad coefficients, normalized by a0 (float64)."""
    g = min(max(gain, 0.1), 10.0)
    w0 = 2.0 * math.pi * center_freq / SAMPLE_RATE
    A = math.exp(g / 40.0 * math.log(10.0))
    alpha = math.sin(w0) / (2.0 * q)
    b0 = 1.0 + alpha * A
    b1 = -2.0 * math.cos(w0)
    b2 = 1.0 - alpha * A
    a0 = 1.0 + alpha / A
    a1 = b1
    a2 = 1.0 - alpha / A
    return b0 / a0, b1 / a0, b2 / a0, a1 / a0, a2 / a0


def _impulse_response(center_freq: float, q: float, gain: float, n: int = 256):
    b0, b1, b2, a1, a2 = _filter_coeffs(center_freq, q, gain)
    h = np.zeros(n, dtype=np.float64)
    x1 = x2 = y1 = y2 = 0.0
    for i in range(n):
        xn = 1.0 if i == 0 else 0.0
        yn = b0 * xn + b1 * x1 + b2 * x2 - a1 * y1 - a2 * y2
        x2, x1 = x1, xn
        y2, y1 = y1, yn
        h[i] = yn
    return h


def _toeplitz_mats(h: np.ndarray):
    """T0T[j,g] = h[g-j] (g>=j else 0); T1T[j,g] = h[128+g-j].  Stationary
    matmul operands, contracting over partition j."""
    j = np.arange(L)[:, None]
    g = np.arange(L)[None, :]
    d0 = g - j
    t0t = np.where(d0 >= 0, h[np.clip(d0, 0, len(h) - 1)], 0.0)
    d1 = 128 + g - j
    t1t = h[np.clip(d1, 0, len(h) - 1)]
    return t0t.astype(np.float32), t1t.astype(np.float32)


_NC_CACHE = {}


def _build_nc():
    """Per-core Bass program (same NEFF on all cores).

    dram x0: [128, WBYTES + SROW]  = [T1|T0 | slab-0 bytes]
    dram xs: [NSLAB-1, 128, SROW]    (slabs 1..3, each with own halo col)
    dram y:  [NSLAB, 128, NU*QW]     (slab-major, units side by side)

    Slab tile layout [128, SROW] (+WBYTES Toeplitz prefix on slab 0):
      unit u's data at byte 513u; col 512q+c'-1 at +c' (c'=0 = halo col).

    Dispatch plan: all loads first on the SP+GpSimd rings (slab 0 split in
    small leading pieces so the MM stream starts early), then slab-major
    compute with stores split on SP+GpSimd.
    """
    import concourse.bass as bass
    import concourse.mybir as mybir
    import concourse.tile as tile
    from concourse.ap import AP

    f8 = mybir.dt.float8e4
    f32 = mybir.dt.float32
    nc = bass.Bass("TRN2")

    x0 = nc.dram_tensor("x0", [L, WBYTES + SROW], f8, kind="ExternalInput")
    xs = nc.dram_tensor("xs", [NSLAB - 1, L, SROW], f8, kind="ExternalInput")
    y = nc.dram_tensor("y", [NSLAB, L, NU * QW], f8, kind="ExternalOutput")
    xs_r = xs[:]
    y_r = y[:]

    QTR = SROW // 4   # 2052 B = 4 units
    ORIG = WBYTES     # slab data offset within the slab-0 tile
    with tile.TileContext(nc) as tc:
        with (
            tc.tile_pool(name="xt", bufs=1) as xt_pool,
            tc.tile_pool(name="yo", bufs=1) as yo_pool,
            tc.tile_pool(name="ps", bufs=7, space="PSUM") as ps_pool,
            tc.tile_pool(name="psw", bufs=1, space="PSUM") as psw_pool,
        ):
            xts = [
                xt_pool.tile(
                    [L, SROW + (WBYTES if q == 0 else 0)], f8, name=f"xt{q}"
                )
                for q in range(NSLAB)
            ]
            yos = [
                yo_pool.tile([L, NU * QW], f8, name=f"yo{q}")
                for q in range(NSLAB)
            ]

            # PE pstate warmup: dummy matmuls on scratch keep the PE busy
            # through the load window so real MMs start at full clock.
            scr = xt_pool.tile([L, L + QW], f8, name="scratch")
            nc.gpsimd.memset(scr[:], 0)
            ps_w = psw_pool.tile([L, QW], f32, name="ps_warm")
            for _ in range(5):
                nc.tensor.matmul(
                    ps_w[:], scr[:, 0:L], scr[:, L : L + QW],
                    start=True, stop=True, skip_group_check=True,
                )

            # --- loads (SP + GpSimd rings) ---
            # slab 0 pieces sized for a fast MM start; weights ride piece 0.
            UB = 513  # bytes per unit
            p0 = WBYTES + 2 * UB
            nc.sync.dma_start(xts[0][:, 0 : p0], x0[:, 0:p0])
            nc.gpsimd.dma_start(
                xts[0][:, p0 : p0 + 2 * UB], x0[:, p0 : p0 + 2 * UB]
            )
            nc.sync.dma_start(
                xts[0][:, ORIG + QTR : ORIG + 2 * QTR],
                x0[:, WBYTES + QTR : WBYTES + 2 * QTR],
            )
            nc.gpsimd.dma_start(
                xts[0][:, ORIG + 2 * QTR : ORIG + 3 * QTR],
                x0[:, WBYTES + 2 * QTR : WBYTES + 3 * QTR],
            )
            nc.sync.dma_start(
                xts[0][:, ORIG + 3 * QTR :], x0[:, WBYTES + 3 * QTR :]
            )
            half = SROW // 2
            for q in range(1, NSLAB):
                nc.sync.dma_start(
                    xts[q][:, 0:half], xs_r[q - 1][:, 0:half]
                )
                nc.gpsimd.dma_start(
                    xts[q][:, half:], xs_r[q - 1][:, half:]
                )

            t1s = xts[0][:, 0:L]
            t0s = xts[0][:, L : 2 * L]

            # --- compute, slab-major; two plain fp8 matmuls per chunk ---
            for q in range(NSLAB):
                xt = xts[q]
                yo = yos[q]
                base = ORIG if q == 0 else 0
                for u in range(NU):
                    ps = ps_pool.tile([L, QW], f32, tag="mm")
                    c0 = base + 513 * u + 1  # byte of X[512q] for unit u
                    nc.tensor.matmul(
                        ps[:], t0s, xt[:, c0 : c0 + QW],
                        start=True, stop=False,
                    )
                    nc.tensor.matmul(
                        ps[:], t1s, xt[:, c0 - 1 : c0 + QW - 1],
                        start=False, stop=True,
                    )
                    dst = yo[:, QW * u : QW * (u + 1)]
                    if u % 2 == 0:
                        nc.vector.tensor_copy(dst, ps[:])
                    else:
                        nc.scalar.copy(dst, ps[:])

                # store unit-group pieces as they finish; finer on the last
                # slab to shorten the tail, coarser early (fewer semaphores
                # shortens the end-of-program wait drain).
                nparts = 4 if q < NSLAB - 1 else 8
                sq = NU * QW // nparts
                for k in range(nparts):
                    ring = nc.sync if k % 2 == 0 else nc.gpsimd
                    ring.dma_start(
                        y_r[q][:, k * sq : (k + 1) * sq],
                        yo[:, k * sq : (k + 1) * sq],
                    )

    return nc


def _strip_redundant_waits(bir_bytes: bytes) -> bytes:
    """PE Matmult/Ldweights lower to TPB instructions with a single
    semaphore-wait slot, but Tile's slot-release deps put 2 waits (old-writer
    PE completion + old-reader DVE completion) on the first toucher of every
    reused PSUM slot.  The PE wait is transitively implied: the DVE evac copy
    whose completion the instruction also waits on had itself waited on those
    PE completions.  Prove the implication with a completion-guarantee
    dataflow (rules: an instruction completes only after its waits hold; TPB
    engine queues are in-order FIFO; a semaphore's v-th update implies its
    earlier updates) and drop provably-redundant waits; raise if a >1-wait
    matmul can't be reduced."""
    import json

    bir = json.loads(bir_bytes)
    insts = []
    containers = []  # (list, index) for each inst, for NoOp insertion

    def walk(block):
        lst = block.get("instructions", [])
        for idx, i in enumerate(lst):
            insts.append(i)
            containers.append((lst, idx))
        for sub in block.get("blocks", []):
            walk(sub)

    for b in bir["functions"][0]["blocks"]:
        walk(b)

    # Per-sem update timeline: list of (cumulative_value, inst_idx).
    timelines = {}
    for k, i in enumerate(insts):
        for u in i.get("sync_info", {}).get("on_update", []) or []:
            if u.get("sync_type") != "semaphore":
                continue
            tl = timelines.setdefault(u["ant_name"], [])
            prev = tl[-1][0] if tl else 0
            tl.append((prev + int(u.get("update_value", 1)), k))

    def producer(sem, val):
        """Index of the instruction whose update first brings sem >= val."""
        tl = timelines.get(sem)
        if not tl:
            return None
        import bisect
        pos = bisect.bisect_left(tl, (val, -1))
        if pos == len(tl):
            return None
        return tl[pos][1]

    IN_ORDER_ENGINES = {"PE", "DVE", "Activation", "Pool", "SP"}
    NOT_IN_ORDER_OPCODES = {"DMACopy", "DmaTransposeAnt"}  # complete out-of-band

    # guarantees[k]: sem -> max value known to hold when inst k completes.
    guarantees = [dict() for _ in insts]
    prev_by_engine = {}
    preds = []  # per-inst: same-engine predecessor (in-order engines only)
    for k, i in enumerate(insts):
        eng = i.get("engine")
        in_order = eng in IN_ORDER_ENGINES and i.get("opcode") not in NOT_IN_ORDER_OPCODES
        pred = prev_by_engine.get(eng) if in_order else None
        preds.append(pred)
        if in_order:
            prev_by_engine[eng] = k

    def merge(dst, src):
        changed = False
        for s, v in src.items():
            if dst.get(s, 0) < v:
                dst[s] = v
                changed = True
        return changed

    for _pass in range(3):
        changed = False
        for k, i in enumerate(insts):
            g = guarantees[k]
            si = i.get("sync_info", {})
            for w in si.get("on_wait", []) or []:
                if w.get("sync_type") != "semaphore":
                    continue
                v = int(w["wait_value"])
                if g.get(w["ant_name"], 0) < v:
                    g[w["ant_name"]] = v
                    changed = True
                p = producer(w["ant_name"], v)
                if p is not None:
                    changed |= merge(g, guarantees[p])
            if preds[k] is not None:
                changed |= merge(g, guarantees[preds[k]])
        # Own updates fire at completion; same-sem update chains are FIFO
        # (engine queue or DMA queue), so the v-th updater inherits the
        # (v-1)-th updater's guarantees.
        for sem, tl in timelines.items():
            prev_idx = None
            for cum, k in tl:
                if guarantees[k].get(sem, 0) < cum:
                    guarantees[k][sem] = cum
                    changed = True
                if prev_idx is not None:
                    changed |= merge(guarantees[k], guarantees[prev_idx])
                prev_idx = k
        if not changed:
            break

    STRIP_OPCODES = {
        "Matmult", "Ldweights", "TensorCopy", "Memset", "DMACopy",
        "DmaTransposeAnt",
        "Activation", "TensorScalarAffineSelect", "TensorTensor",
        "TensorScalarPtr", "TensorReduce", "Drain", "NoOp",
    }
    stripped = 0
    inserts = []  # (list, index, [noop dicts])
    for k, i in enumerate(insts):
        if i.get("opcode") not in STRIP_OPCODES:
            continue
        si = i.get("sync_info", {})
        waits = si.get("on_wait", []) or []
        if len(waits) <= 1:
            continue
        # Drop every wait implied by another (not-yet-dropped) wait's
        # producer guarantee.
        kept = list(waits)
        changed = True
        while changed:
            changed = False
            for w in list(kept):
                if len(kept) == 1:
                    break
                for w2 in kept:
                    if w2 is w:
                        continue
                    p = producer(w2["ant_name"], int(w2["wait_value"]))
                    if p is not None and guarantees[p].get(w["ant_name"], 0) >= int(
                        w["wait_value"]
                    ):
                        kept.remove(w)
                        changed = True
                        break
        stripped += len(waits) - len(kept)
        si["on_wait"] = [kept[-1]]
        if len(kept) > 1:
            # Split remaining waits onto single-wait NoOps ahead of the
            # instruction on the same engine queue.
            lst, idx = containers[k]
            noops = [
                {
                    "debug": i.get("debug", 0),
                    "engine": i.get("engine"),
                    "ins": [],
                    "name": f"{i['name']}-w{j}",
                    "opcode": "NoOp",
                    "outs": [],
                    "sync_info": {"on_wait": [w], "on_update": []},
                }
                for j, w in enumerate(kept[:-1])
            ]
            inserts.append((lst, idx, noops))

    # Apply insertions (descending index per list keeps positions valid).
    from collections import defaultdict
    by_list = defaultdict(list)
    for lst, idx, noops in inserts:
        by_list[id(lst)].append((lst, idx, noops))
    for entries in by_list.values():
        for lst, idx, noops in sorted(entries, key=lambda e: -e[1]):
            lst[idx:idx] = noops

    out = json.dumps(bir).encode()
    return out


def audit_waits(bir_bytes):
    """Flag instructions with more than the single hardware wait slot."""
    import json

    bir = json.loads(bir_bytes)
    checked = {
        "Matmult", "Ldweights", "TensorCopy", "Memset", "DMACopy",
        "DmaTransposeAnt",
        "Activation", "TensorScalarAffineSelect", "TensorTensor",
        "TensorScalarPtr", "TensorReduce",
    }
    bad = []
    def walk(block):
        for i in block.get("instructions", []):
            if i.get("opcode") not in checked:
                continue
            w = i.get("sync_info", {}).get("on_wait", [])
            if len(w) > 1:
                bad.append((i["name"], i.get("opcode"), i.get("engine"),
                            [(x["ant_name"], x["wait_value"]) for x in w]))
        for sub in block.get("blocks", []):
            walk(sub)
    for b in bir["functions"][0]["blocks"]:
        walk(b)
    return bad


def _get_nc():
    if "nc" not in _NC_CACHE:
        nc = _build_nc()
        patched = _strip_redundant_waits(type(nc).to_json_bytes(nc))
        bad = audit_waits(patched)
        if bad:
            raise RuntimeError(f"multi-wait instructions remain: {bad[:5]}")
        nc.to_json_bytes = lambda: patched
        _NC_CACHE["nc"] = nc
    return _NC_CACHE["nc"]


def _host_prep(x64_f32: np.ndarray, t0t: np.ndarray, t1t: np.ndarray):
    """fp32 [64, T] -> per-core in_maps with fp8 interleaved slab layout."""
    import ml_dtypes

    f8 = ml_dtypes.float8_e4m3fn
    # block-major X'[s, j, B] then fp8
    xbm = np.ascontiguousarray(
        x64_f32.reshape(SIGS, NBLK, L).transpose(0, 2, 1)
    ).astype(f8)
    w8 = np.concatenate([t1t, t0t], axis=1).astype(f8)  # [128, 256] (T1|T0)

    in_maps = []
    for c in range(N_CORES):
        sig = xbm[SPC * c : SPC * (c + 1)]             # [8, 128, 4096]
        units = sig.reshape(SPC, L, 2, UW).transpose(0, 2, 1, 3).reshape(
            NU, L, UW
        )                                               # u = 2s+h
        # halo col per (slab, unit): col 512q-1 of the unit (zeros for
        # q==0 & first-half units; second-half q==0 halo = col 2047 of
        # the signal's first half = units[u-1][:, -1]).
        slabs = np.zeros((NSLAB, L, SROW), dtype=f8)
        for q in range(NSLAB):
            halo = np.zeros((NU, L), dtype=f8)
            if q == 0:
                halo[1::2] = units[0::2, :, UW - 1]
            else:
                halo[:] = units[:, :, QW * q - 1]
            block = units[:, :, QW * q : QW * (q + 1)]  # [16, 128, 512]
            unit_rows = np.concatenate(
                [halo[:, :, None], block], axis=2
            )  # [16, 128, 513]: unit-major, halo col first
            slabs[q] = np.ascontiguousarray(
                unit_rows.transpose(1, 0, 2)
            ).reshape(L, SROW)
        x0 = np.ascontiguousarray(
            np.concatenate([w8, slabs[0]], axis=1)
        )
        in_maps.append({"x0": x0, "xs": np.ascontiguousarray(slabs[1:])})
    return in_maps


def _host_finish(results, x_f32: np.ndarray):
    """Per-core y slabs -> d [64, T] fp32; return y = x + d."""
    d64 = np.empty((SIGS, T_FULL), dtype=np.float32)
    for c in range(N_CORES):
        ys = np.asarray(results[c]["y"]).astype(np.float32)  # [4, 128, 8192]
        # ys[q][p][512u+j] = unit u col 512q+j
        units = ys.reshape(NSLAB, L, NU, QW).transpose(2, 1, 0, 3).reshape(
            NU, L, UW
        )
        sig = units.reshape(SPC, 2, L, UW).transpose(0, 2, 1, 3).reshape(
            SPC, L, NBLK
        )
        d64[SPC * c : SPC * (c + 1)] = (
            sig.transpose(0, 2, 1).reshape(SPC, T_FULL)
        )
    return x_f32 + d64


def run_spmd(x64: np.ndarray, t0t: np.ndarray, t1t: np.ndarray, trace: bool = False):
    """x64: [64, T] float32 -> [64, T] float32 (plus BassKernelResults)."""
    from concourse.bass_utils import run_bass_kernel_spmd

    nc = _get_nc()
    in_maps = _host_prep(x64, t0t, t1t)
    res = run_bass_kernel_spmd(
        nc, in_maps, core_ids=list(range(N_CORES)), trace=trace
    )
    out = _host_finish(res.results, x64)
    return out, res


def kernel(x, center_freq, q, gain, t=0, **_unused):
    x = np.ascontiguousarray(np.asarray(x), dtype=np.float32)
    assert x.shape == (B_FULL, C_FULL, T_FULL), x.shape
    cf = float(np.asarray(center_freq).reshape(-1)[0])
    qv = float(np.asarray(q).reshape(-1)[0])
    gv = float(np.asarray(gain).reshape(-1)[0])

    h = _impulse_response(cf, qv, gv)
    h[0] -= 1.0  # residual filter: d = y - x
    t0t, t1t = _toeplitz_mats(h)

    x64 = x.reshape(SIGS, T_FULL)
    out, _ = run_spmd(x64, t0t, t1t, trace=False)
    return out.reshape(B_FULL, C_FULL, T_FULL).astype(np.float32)


# revision 5
# speedup vs baseline: 1.0058x; 1.0058x over previous
"""Biquad peaking-EQ IIR on 8 Trainium2 NeuronCores — fp8 residual edition.

Math: the reference filter y = H(x) is a 2nd-order IIR whose impulse response
decays below 1e-10 after 256 taps; we compute the *residual* d = y - x as a
truncated-FIR convolution (taps g = h - delta), quantize everything to
fp8 e4m3, and reconstruct y = x + d on the host (which already holds x in
fp32).  End-to-end rel-L2 error ~6e-3 (numpy-validated), under the 2e-2 gate.

Performance design (59.7us bf16 ancestor -> ~45.5us measured):
 1. fp8 e4m3 input AND output halve DMA bytes to 8.4 MB/core -> ~24us of
    HBM flow at the ~350 B/ns per-core aggregate rate.
 2. Two plain fp8 matmuls per 512-block chunk (T0 x X[c] + T1 x X[c-1],
    fp32 PSUM accumulation); back-to-back MMs issue every ~216 ns (full
    2.4 GHz), so the 128-MM PE stream is ~27.6us and is the critical
    stream.  (fp8 DoubleRow would halve MM count but requires the moving
    k-pair stride to be a multiple of 16 bytes: overlapping stride-1
    views crash the PE, a 16-byte-interleaved layout runs the strided
    fetch 2.8x slow, and building an on-chip shifted duplicate costs more
    than it saves - SBUF->SBUF DMA measured ~163-260 B/ns with multi-us
    completion lag, and engine copies are parity-blocked on the odd
    1-byte shift.  All measured; plain wins.)
 3. Work streams in 4 column-slabs (512 block cols x 16 half-signal
    units, ~1.05 MB).  Each slab tile carries its own leading halo col
    per unit (host-duplicated), so slabs are fully independent; loads and
    stores split across the SP and GpSimd HWDGE rings (2 queues x ~200
    B/ns covers the ~350 B/ns core cap); compute is slab-major so output
    retires evenly; stores go out in unit-group quarters (eighths on the
    last slab) right behind the evacuations to minimize the tail.
 4. PSUM chunk tiles [128,512] f32 (7 bufs); evacuation (fp32->fp8 cast,
    ~687ns/chunk) alternates DVE/ACT per unit (~22us each, hidden under
    the PE stream).  The Toeplitz pair rides the front of slab 0's first
    load piece; slab 0 loads in small pieces so the first MM fires ~10.5us.
 5. PE pstate warmup: 5 dummy matmuls on a zeroed scratch tile keep the
    PE busy from ~8.3us so real MMs reach the full clock quickly (the
    PE downclocks unless continuously busy ~3us).

Scheduling note: TPB instructions have a single semaphore-wait slot; Tile's
slot-release deps routinely emit 2+ waits.  _strip_redundant_waits
post-processes the scheduled BIR (see its docstring).
"""

import math

import numpy as np

SAMPLE_RATE = 44100.0

# Problem geometry (hardcoded per harness contract).
B_FULL, C_FULL, T_FULL = 32, 2, 524288
N_CORES = 8
SIGS = B_FULL * C_FULL          # 64 signals
SPC = SIGS // N_CORES           # 8 signals per core
L = 128                         # block size == PE array dim
NBLK = T_FULL // L              # 4096 block cols per signal
NU = 16                         # interleaved half-signal units per core
UW = NBLK // 2                  # 2048 block cols per unit
QW = 512                        # block cols per chunk (= PSUM bank, fp32)
NSLAB = UW // QW                # 4 slabs
SROW = NU * (1 + QW)            # slab row bytes: 16 units x (halo col + 512)
WBYTES = 2 * L                  # Toeplitz pair rides the front of slab 0


def _filter_coeffs(center_freq: float, q: float, gain: float):
    """torchaudio equalizer_biquad coefficients, normalized by a0 (float64)."""
    g = min(max(gain, 0.1), 10.0)
    w0 = 2.0 * math.pi * center_freq / SAMPLE_RATE
    A = math.exp(g / 40.0 * math.log(10.0))
    alpha = math.sin(w0) / (2.0 * q)
    b0 = 1.0 + alpha * A
    b1 = -2.0 * math.cos(w0)
    b2 = 1.0 - alpha * A
    a0 = 1.0 + alpha / A
    a1 = b1
    a2 = 1.0 - alpha / A
    return b0 / a0, b1 / a0, b2 / a0, a1 / a0, a2 / a0


def _impulse_response(center_freq: float, q: float, gain: float, n: int = 256):
    b0, b1, b2, a1, a2 = _filter_coeffs(center_freq, q, gain)
    h = np.zeros(n, dtype=np.float64)
    x1 = x2 = y1 = y2 = 0.0
    for i in range(n):
        xn = 1.0 if i == 0 else 0.0
        yn = b0 * xn + b1 * x1 + b2 * x2 - a1 * y1 - a2 * y2
        x2, x1 = x1, xn
        y2, y1 = y1, yn
        h[i] = yn
    return h


def _toeplitz_mats(h: np.ndarray):
    """T0T[j,g] = h[g-j] (g>=j else 0); T1T[j,g] = h[128+g-j].  Stationary
    matmul operands, contracting over partition j."""
    j = np.arange(L)[:, None]
    g = np.arange(L)[None, :]
    d0 = g - j
    t0t = np.where(d0 >= 0, h[np.clip(d0, 0, len(h) - 1)], 0.0)
    d1 = 128 + g - j
    t1t = h[np.clip(d1, 0, len(h) - 1)]
    return t0t.astype(np.float32), t1t.astype(np.float32)


_NC_CACHE = {}


def _build_nc():
    """Per-core Bass program (same NEFF on all cores).

    dram x0: [128, WBYTES + SROW]  = [T1|T0 | slab-0 bytes]
    dram xs: [NSLAB-1, 128, SROW]    (slabs 1..3, each with own halo col)
    dram y:  [NSLAB, 128, NU*QW]     (slab-major, units side by side)

    Slab tile layout [128, SROW] (+WBYTES Toeplitz prefix on slab 0):
      unit u's data at byte 513u; col 512q+c'-1 at +c' (c'=0 = halo col).

    Dispatch plan: all loads first on the SP+GpSimd rings (slab 0 split in
    small leading pieces so the MM stream starts early), then slab-major
    compute with stores split on SP+GpSimd.
    """
    import concourse.bass as bass
    import concourse.mybir as mybir
    import concourse.tile as tile
    from concourse.ap import AP

    f8 = mybir.dt.float8e4
    f32 = mybir.dt.float32
    nc = bass.Bass("TRN2")

    x0 = nc.dram_tensor("x0", [L, WBYTES + SROW], f8, kind="ExternalInput")
    xs = nc.dram_tensor("xs", [NSLAB - 1, L, SROW], f8, kind="ExternalInput")
    y = nc.dram_tensor("y", [NSLAB, L, NU * QW], f8, kind="ExternalOutput")
    xs_r = xs[:]
    y_r = y[:]

    QTR = SROW // 4   # 2052 B = 4 units
    ORIG = WBYTES     # slab data offset within the slab-0 tile
    with tile.TileContext(nc) as tc:
        with (
            tc.tile_pool(name="xt", bufs=1) as xt_pool,
            tc.tile_pool(name="yo", bufs=1) as yo_pool,
            tc.tile_pool(name="ps", bufs=7, space="PSUM") as ps_pool,
            tc.tile_pool(name="psw", bufs=1, space="PSUM") as psw_pool,
        ):
            xts = [
                xt_pool.tile(
                    [L, SROW + (WBYTES if q == 0 else 0)], f8, name=f"xt{q}"
                )
                for q in range(NSLAB)
            ]
            yos = [
                yo_pool.tile([L, NU * QW], f8, name=f"yo{q}")
                for q in range(NSLAB)
            ]

            # PE pstate warmup: dummy matmuls on scratch keep the PE busy
            # through the load window so real MMs start at full clock.
            scr = xt_pool.tile([L, L + QW], f8, name="scratch")
            nc.gpsimd.memset(scr[:], 0)
            ps_w = psw_pool.tile([L, QW], f32, name="ps_warm")
            for _ in range(4):
                nc.tensor.matmul(
                    ps_w[:], scr[:, 0:L], scr[:, L : L + QW],
                    start=True, stop=True, skip_group_check=True,
                )

            # --- loads (SP + GpSimd rings) ---
            # slab 0 in geometrically growing pieces (W+u0, u1, u2-3, u4-7,
            # u8-15) so the first MM fires as early as possible.
            UB = 513  # bytes per unit
            cuts = [0, WBYTES + UB, WBYTES + 2 * UB, WBYTES + 4 * UB,
                    WBYTES + 8 * UB, WBYTES + 12 * UB, WBYTES + 16 * UB]
            for k in range(6):
                ring = nc.sync if k % 2 == 0 else nc.gpsimd
                ring.dma_start(
                    xts[0][:, cuts[k] : cuts[k + 1]],
                    x0[:, cuts[k] : cuts[k + 1]],
                )
            half = SROW // 2
            for q in range(1, NSLAB):
                nc.sync.dma_start(
                    xts[q][:, 0:half], xs_r[q - 1][:, 0:half]
                )
                nc.gpsimd.dma_start(
                    xts[q][:, half:], xs_r[q - 1][:, half:]
                )

            t1s = xts[0][:, 0:L]
            t0s = xts[0][:, L : 2 * L]

            # --- compute, slab-major; two plain fp8 matmuls per chunk ---
            for q in range(NSLAB):
                xt = xts[q]
                yo = yos[q]
                base = ORIG if q == 0 else 0
                for u in range(NU):
                    ps = ps_pool.tile([L, QW], f32, tag="mm")
                    c0 = base + 513 * u + 1  # byte of X[512q] for unit u
                    nc.tensor.matmul(
                        ps[:], t0s, xt[:, c0 : c0 + QW],
                        start=True, stop=False,
                    )
                    nc.tensor.matmul(
                        ps[:], t1s, xt[:, c0 - 1 : c0 + QW - 1],
                        start=False, stop=True,
                    )
                    dst = yo[:, QW * u : QW * (u + 1)]
                    if q == NSLAB - 1 and u >= 14:
                        # tail: halve the last evacs across both engines
                        nc.vector.tensor_copy(dst[:, 0 : QW // 2],
                                              ps[:, 0 : QW // 2])
                        nc.scalar.copy(dst[:, QW // 2 :], ps[:, QW // 2 :])
                    elif u % 2 == 0:
                        nc.vector.tensor_copy(dst, ps[:])
                    else:
                        nc.scalar.copy(dst, ps[:])

                # store unit-group pieces as they finish; finer on the last
                # slab to shorten the tail, coarser early (fewer semaphores
                # shortens the end-of-program wait drain).
                nparts = 4 if q < NSLAB - 1 else 8
                sq = NU * QW // nparts
                for k in range(nparts):
                    ring = nc.sync if k % 2 == 0 else nc.gpsimd
                    ring.dma_start(
                        y_r[q][:, k * sq : (k + 1) * sq],
                        yo[:, k * sq : (k + 1) * sq],
                    )

    return nc


def _strip_redundant_waits(bir_bytes: bytes) -> bytes:
    """PE Matmult/Ldweights lower to TPB instructions with a single
    semaphore-wait slot, but Tile's slot-release deps put 2 waits (old-writer
    PE completion + old-reader DVE completion) on the first toucher of every
    reused PSUM slot.  The PE wait is transitively implied: the DVE evac copy
    whose completion the instruction also waits on had itself waited on those
    PE completions.  Prove the implication with a completion-guarantee
    dataflow (rules: an instruction completes only after its waits hold; TPB
    engine queues are in-order FIFO; a semaphore's v-th update implies its
    earlier updates) and drop provably-redundant waits; raise if a >1-wait
    matmul can't be reduced."""
    import json

    bir = json.loads(bir_bytes)
    insts = []
    containers = []  # (list, index) for each inst, for NoOp insertion

    def walk(block):
        lst = block.get("instructions", [])
        for idx, i in enumerate(lst):
            insts.append(i)
            containers.append((lst, idx))
        for sub in block.get("blocks", []):
            walk(sub)

    for b in bir["functions"][0]["blocks"]:
        walk(b)

    # Per-sem update timeline: list of (cumulative_value, inst_idx).
    timelines = {}
    for k, i in enumerate(insts):
        for u in i.get("sync_info", {}).get("on_update", []) or []:
            if u.get("sync_type") != "semaphore":
                continue
            tl = timelines.setdefault(u["ant_name"], [])
            prev = tl[-1][0] if tl else 0
            tl.append((prev + int(u.get("update_value", 1)), k))

    def producer(sem, val):
        """Index of the instruction whose update first brings sem >= val."""
        tl = timelines.get(sem)
        if not tl:
            return None
        import bisect
        pos = bisect.bisect_left(tl, (val, -1))
        if pos == len(tl):
            return None
        return tl[pos][1]

    IN_ORDER_ENGINES = {"PE", "DVE", "Activation", "Pool", "SP"}
    NOT_IN_ORDER_OPCODES = {"DMACopy", "DmaTransposeAnt"}  # complete out-of-band

    # guarantees[k]: sem -> max value known to hold when inst k completes.
    guarantees = [dict() for _ in insts]
    prev_by_engine = {}
    preds = []  # per-inst: same-engine predecessor (in-order engines only)
    for k, i in enumerate(insts):
        eng = i.get("engine")
        in_order = eng in IN_ORDER_ENGINES and i.get("opcode") not in NOT_IN_ORDER_OPCODES
        pred = prev_by_engine.get(eng) if in_order else None
        preds.append(pred)
        if in_order:
            prev_by_engine[eng] = k

    def merge(dst, src):
        changed = False
        for s, v in src.items():
            if dst.get(s, 0) < v:
                dst[s] = v
                changed = True
        return changed

    for _pass in range(3):
        changed = False
        for k, i in enumerate(insts):
            g = guarantees[k]
            si = i.get("sync_info", {})
            for w in si.get("on_wait", []) or []:
                if w.get("sync_type") != "semaphore":
                    continue
                v = int(w["wait_value"])
                if g.get(w["ant_name"], 0) < v:
                    g[w["ant_name"]] = v
                    changed = True
                p = producer(w["ant_name"], v)
                if p is not None:
                    changed |= merge(g, guarantees[p])
            if preds[k] is not None:
                changed |= merge(g, guarantees[preds[k]])
        # Own updates fire at completion; same-sem update chains are FIFO
        # (engine queue or DMA queue), so the v-th updater inherits the
        # (v-1)-th updater's guarantees.
        for sem, tl in timelines.items():
            prev_idx = None
            for cum, k in tl:
                if guarantees[k].get(sem, 0) < cum:
                    guarantees[k][sem] = cum
                    changed = True
                if prev_idx is not None:
                    changed |= merge(guarantees[k], guarantees[prev_idx])
                prev_idx = k
        if not changed:
            break

    STRIP_OPCODES = {
        "Matmult", "Ldweights", "TensorCopy", "Memset", "DMACopy",
        "DmaTransposeAnt",
        "Activation", "TensorScalarAffineSelect", "TensorTensor",
        "TensorScalarPtr", "TensorReduce", "Drain", "NoOp",
    }
    stripped = 0
    inserts = []  # (list, index, [noop dicts])
    for k, i in enumerate(insts):
        if i.get("opcode") not in STRIP_OPCODES:
            continue
        si = i.get("sync_info", {})
        waits = si.get("on_wait", []) or []
        if len(waits) <= 1:
            continue
        # Drop every wait implied by another (not-yet-dropped) wait's
        # producer guarantee.
        kept = list(waits)
        changed = True
        while changed:
            changed = False
            for w in list(kept):
                if len(kept) == 1:
                    break
                for w2 in kept:
                    if w2 is w:
                        continue
                    p = producer(w2["ant_name"], int(w2["wait_value"]))
                    if p is not None and guarantees[p].get(w["ant_name"], 0) >= int(
                        w["wait_value"]
                    ):
                        kept.remove(w)
                        changed = True
                        break
        stripped += len(waits) - len(kept)
        si["on_wait"] = [kept[-1]]
        if len(kept) > 1:
            # Split remaining waits onto single-wait NoOps ahead of the
            # instruction on the same engine queue.
            lst, idx = containers[k]
            noops = [
                {
                    "debug": i.get("debug", 0),
                    "engine": i.get("engine"),
                    "ins": [],
                    "name": f"{i['name']}-w{j}",
                    "opcode": "NoOp",
                    "outs": [],
                    "sync_info": {"on_wait": [w], "on_update": []},
                }
                for j, w in enumerate(kept[:-1])
            ]
            inserts.append((lst, idx, noops))

    # Apply insertions (descending index per list keeps positions valid).
    from collections import defaultdict
    by_list = defaultdict(list)
    for lst, idx, noops in inserts:
        by_list[id(lst)].append((lst, idx, noops))
    for entries in by_list.values():
        for lst, idx, noops in sorted(entries, key=lambda e: -e[1]):
            lst[idx:idx] = noops

    out = json.dumps(bir).encode()
    return out


def audit_waits(bir_bytes):
    """Flag instructions with more than the single hardware wait slot."""
    import json

    bir = json.loads(bir_bytes)
    checked = {
        "Matmult", "Ldweights", "TensorCopy", "Memset", "DMACopy",
        "DmaTransposeAnt",
        "Activation", "TensorScalarAffineSelect", "TensorTensor",
        "TensorScalarPtr", "TensorReduce",
    }
    bad = []
    def walk(block):
        for i in block.get("instructions", []):
            if i.get("opcode") not in checked:
                continue
            w = i.get("sync_info", {}).get("on_wait", [])
            if len(w) > 1:
                bad.append((i["name"], i.get("opcode"), i.get("engine"),
                            [(x["ant_name"], x["wait_value"]) for x in w]))
        for sub in block.get("blocks", []):
            walk(sub)
    for b in bir["functions"][0]["blocks"]:
        walk(b)
    return bad


def _get_nc():
    if "nc" not in _NC_CACHE:
        nc = _build_nc()
        patched = _strip_redundant_waits(type(nc).to_json_bytes(nc))
        bad = audit_waits(patched)
        if bad:
            raise RuntimeError(f"multi-wait instructions remain: {bad[:5]}")
        nc.to_json_bytes = lambda: patched
        _NC_CACHE["nc"] = nc
    return _NC_CACHE["nc"]


def _host_prep(x64_f32: np.ndarray, t0t: np.ndarray, t1t: np.ndarray):
    """fp32 [64, T] -> per-core in_maps with fp8 interleaved slab layout."""
    import ml_dtypes

    f8 = ml_dtypes.float8_e4m3fn
    # block-major X'[s, j, B] then fp8
    xbm = np.ascontiguousarray(
        x64_f32.reshape(SIGS, NBLK, L).transpose(0, 2, 1)
    ).astype(f8)
    w8 = np.concatenate([t1t, t0t], axis=1).astype(f8)  # [128, 256] (T1|T0)

    in_maps = []
    for c in range(N_CORES):
        sig = xbm[SPC * c : SPC * (c + 1)]             # [8, 128, 4096]
        units = sig.reshape(SPC, L, 2, UW).transpose(0, 2, 1, 3).reshape(
            NU, L, UW
        )                                               # u = 2s+h
        # halo col per (slab, unit): col 512q-1 of the unit (zeros for
        # q==0 & first-half units; second-half q==0 halo = col 2047 of
        # the signal's first half = units[u-1][:, -1]).
        slabs = np.zeros((NSLAB, L, SROW), dtype=f8)
        for q in range(NSLAB):
            halo = np.zeros((NU, L), dtype=f8)
            if q == 0:
                halo[1::2] = units[0::2, :, UW - 1]
            else:
                halo[:] = units[:, :, QW * q - 1]
            block = units[:, :, QW * q : QW * (q + 1)]  # [16, 128, 512]
            unit_rows = np.concatenate(
                [halo[:, :, None], block], axis=2
            )  # [16, 128, 513]: unit-major, halo col first
            slabs[q] = np.ascontiguousarray(
                unit_rows.transpose(1, 0, 2)
            ).reshape(L, SROW)
        x0 = np.ascontiguousarray(
            np.concatenate([w8, slabs[0]], axis=1)
        )
        in_maps.append({"x0": x0, "xs": np.ascontiguousarray(slabs[1:])})
    return in_maps


def _host_finish(results, x_f32: np.ndarray):
    """Per-core y slabs -> d [64, T] fp32; return y = x + d."""
    d64 = np.empty((SIGS, T_FULL), dtype=np.float32)
    for c in range(N_CORES):
        ys = np.asarray(results[c]["y"]).astype(np.float32)  # [4, 128, 8192]
        # ys[q][p][512u+j] = unit u col 512q+j
        units = ys.reshape(NSLAB, L, NU, QW).transpose(2, 1, 0, 3).reshape(
            NU, L, UW
        )
        sig = units.reshape(SPC, 2, L, UW).transpose(0, 2, 1, 3).reshape(
            SPC, L, NBLK
        )
        d64[SPC * c : SPC * (c + 1)] = (
            sig.transpose(0, 2, 1).reshape(SPC, T_FULL)
        )
    return x_f32 + d64


def run_spmd(x64: np.ndarray, t0t: np.ndarray, t1t: np.ndarray, trace: bool = False):
    """x64: [64, T] float32 -> [64, T] float32 (plus BassKernelResults)."""
    from concourse.bass_utils import run_bass_kernel_spmd

    nc = _get_nc()
    in_maps = _host_prep(x64, t0t, t1t)
    res = run_bass_kernel_spmd(
        nc, in_maps, core_ids=list(range(N_CORES)), trace=trace
    )
    out = _host_finish(res.results, x64)
    return out, res


def kernel(x, center_freq, q, gain, t=0, **_unused):
    x = np.ascontiguousarray(np.asarray(x), dtype=np.float32)
    assert x.shape == (B_FULL, C_FULL, T_FULL), x.shape
    cf = float(np.asarray(center_freq).reshape(-1)[0])
    qv = float(np.asarray(q).reshape(-1)[0])
    gv = float(np.asarray(gain).reshape(-1)[0])

    h = _impulse_response(cf, qv, gv)
    h[0] -= 1.0  # residual filter: d = y - x
    t0t, t1t = _toeplitz_mats(h)

    x64 = x.reshape(SIGS, T_FULL)
    out, _ = run_spmd(x64, t0t, t1t, trace=False)
    return out.reshape(B_FULL, C_FULL, T_FULL).astype(np.float32)


# revision 6
# speedup vs baseline: 1.0213x; 1.0155x over previous
"""Biquad peaking-EQ IIR on 8 Trainium2 NeuronCores — fp8 residual edition.

Math: the reference filter y = H(x) is a 2nd-order IIR whose impulse response
decays below 1e-10 after 256 taps; we compute the *residual* d = y - x as a
truncated-FIR convolution (taps g = h - delta), quantize everything to
fp8 e4m3, and reconstruct y = x + d on the host (which already holds x in
fp32).  End-to-end rel-L2 error ~6e-3 (numpy-validated), under the 2e-2 gate.

Performance design (59.7us bf16 ancestor -> ~45.5us measured):
 1. fp8 e4m3 input AND output halve DMA bytes to 8.4 MB/core -> ~24us of
    HBM flow at the ~350 B/ns per-core aggregate rate.
 2. Two plain fp8 matmuls per 512-block chunk (T0 x X[c] + T1 x X[c-1],
    fp32 PSUM accumulation); back-to-back MMs issue every ~216 ns (full
    2.4 GHz), so the 128-MM PE stream is ~27.6us and is the critical
    stream.  (fp8 DoubleRow would halve MM count but requires the moving
    k-pair stride to be a multiple of 16 bytes: overlapping stride-1
    views crash the PE, a 16-byte-interleaved layout runs the strided
    fetch 2.8x slow, and building an on-chip shifted duplicate costs more
    than it saves - SBUF->SBUF DMA measured ~163-260 B/ns with multi-us
    completion lag, and engine copies are parity-blocked on the odd
    1-byte shift.  All measured; plain wins.)
 3. Work streams in 4 column-slabs (512 block cols x 16 half-signal
    units, ~1.05 MB).  Each slab tile carries its own leading halo col
    per unit (host-duplicated), so slabs are fully independent; loads and
    stores split across the SP and GpSimd HWDGE rings (2 queues x ~200
    B/ns covers the ~350 B/ns core cap); compute is slab-major so output
    retires evenly; stores go out in unit-group quarters (eighths on the
    last slab) right behind the evacuations to minimize the tail.
 4. PSUM chunk tiles [128,512] f32 (7 bufs); evacuation (fp32->fp8 cast,
    ~687ns/chunk) alternates DVE/ACT per unit (~22us each, hidden under
    the PE stream).  The Toeplitz pair rides the front of slab 0's first
    load piece; slab 0 loads in small pieces so the first MM fires ~10.5us.
 5. PE pstate warmup: 5 dummy matmuls on a zeroed scratch tile keep the
    PE busy from ~8.3us so real MMs reach the full clock quickly (the
    PE downclocks unless continuously busy ~3us).

Scheduling note: TPB instructions have a single semaphore-wait slot; Tile's
slot-release deps routinely emit 2+ waits.  _strip_redundant_waits
post-processes the scheduled BIR (see its docstring).
"""

import math

import numpy as np

SAMPLE_RATE = 44100.0

# Problem geometry (hardcoded per harness contract).
B_FULL, C_FULL, T_FULL = 32, 2, 524288
N_CORES = 8
SIGS = B_FULL * C_FULL          # 64 signals
SPC = SIGS // N_CORES           # 8 signals per core
L = 128                         # block size == PE array dim
NBLK = T_FULL // L              # 4096 block cols per signal
NU = 16                         # interleaved half-signal units per core
UW = NBLK // 2                  # 2048 block cols per unit
QW = 512                        # block cols per chunk (= PSUM bank, fp32)
NSLAB = UW // QW                # 4 slabs
SROW = NU * (1 + QW)            # slab row bytes: 16 units x (halo col + 512)
WBYTES = 2 * L                  # Toeplitz pair rides the front of slab 0


def _filter_coeffs(center_freq: float, q: float, gain: float):
    """torchaudio equalizer_biquad coefficients, normalized by a0 (float64)."""
    g = min(max(gain, 0.1), 10.0)
    w0 = 2.0 * math.pi * center_freq / SAMPLE_RATE
    A = math.exp(g / 40.0 * math.log(10.0))
    alpha = math.sin(w0) / (2.0 * q)
    b0 = 1.0 + alpha * A
    b1 = -2.0 * math.cos(w0)
    b2 = 1.0 - alpha * A
    a0 = 1.0 + alpha / A
    a1 = b1
    a2 = 1.0 - alpha / A
    return b0 / a0, b1 / a0, b2 / a0, a1 / a0, a2 / a0


def _impulse_response(center_freq: float, q: float, gain: float, n: int = 256):
    b0, b1, b2, a1, a2 = _filter_coeffs(center_freq, q, gain)
    h = np.zeros(n, dtype=np.float64)
    x1 = x2 = y1 = y2 = 0.0
    for i in range(n):
        xn = 1.0 if i == 0 else 0.0
        yn = b0 * xn + b1 * x1 + b2 * x2 - a1 * y1 - a2 * y2
        x2, x1 = x1, xn
        y2, y1 = y1, yn
        h[i] = yn
    return h


def _toeplitz_mats(h: np.ndarray):
    """T0T[j,g] = h[g-j] (g>=j else 0); T1T[j,g] = h[128+g-j].  Stationary
    matmul operands, contracting over partition j."""
    j = np.arange(L)[:, None]
    g = np.arange(L)[None, :]
    d0 = g - j
    t0t = np.where(d0 >= 0, h[np.clip(d0, 0, len(h) - 1)], 0.0)
    d1 = 128 + g - j
    t1t = h[np.clip(d1, 0, len(h) - 1)]
    return t0t.astype(np.float32), t1t.astype(np.float32)


_NC_CACHE = {}


def _build_nc():
    """Per-core Bass program (same NEFF on all cores).

    dram x0: [128, WBYTES + SROW]  = [T1|T0 | slab-0 bytes]
    dram xs: [NSLAB-1, 128, SROW]    (slabs 1..3, each with own halo col)
    dram y:  [NSLAB, 128, NU*QW]     (slab-major, units side by side)

    Slab tile layout [128, SROW] (+WBYTES Toeplitz prefix on slab 0):
      unit u's data at byte 513u; col 512q+c'-1 at +c' (c'=0 = halo col).

    Dispatch plan: all loads first on the SP+GpSimd rings (slab 0 split in
    small leading pieces so the MM stream starts early), then slab-major
    compute with stores split on SP+GpSimd.
    """
    import concourse.bass as bass
    import concourse.mybir as mybir
    import concourse.tile as tile
    from concourse.ap import AP

    f8 = mybir.dt.float8e4
    f32 = mybir.dt.float32
    nc = bass.Bass("TRN2")

    x0 = nc.dram_tensor("x0", [L, WBYTES + SROW], f8, kind="ExternalInput")
    xs = nc.dram_tensor("xs", [NSLAB - 1, L, SROW], f8, kind="ExternalInput")
    y = nc.dram_tensor("y", [NSLAB, L, NU * QW], f8, kind="ExternalOutput")
    xs_r = xs[:]
    y_r = y[:]

    QTR = SROW // 4   # 2052 B = 4 units
    ORIG = WBYTES     # slab data offset within the slab-0 tile
    with tile.TileContext(nc) as tc:
        with (
            tc.tile_pool(name="xt", bufs=1) as xt_pool,
            tc.tile_pool(name="yo", bufs=1) as yo_pool,
            tc.tile_pool(name="ps", bufs=7, space="PSUM") as ps_pool,
            tc.tile_pool(name="psw", bufs=1, space="PSUM") as psw_pool,
        ):
            xts = [
                xt_pool.tile(
                    [L, SROW + (WBYTES if q == 0 else 0)], f8, name=f"xt{q}"
                )
                for q in range(NSLAB)
            ]
            yos = [
                yo_pool.tile([L, NU * QW], f8, name=f"yo{q}")
                for q in range(NSLAB)
            ]

            # PE pstate warmup: dummy matmuls on scratch keep the PE busy
            # through the load window so real MMs start at full clock.
            scr = xt_pool.tile([L, L + QW], f8, name="scratch")
            nc.gpsimd.memset(scr[:, 0 : (L + QW) // 2], 0)
            nc.vector.memset(scr[:, (L + QW) // 2 :], 0)
            ps_w = psw_pool.tile([L, QW], f32, name="ps_warm")
            for _ in range(4):
                nc.tensor.matmul(
                    ps_w[:], scr[:, 0:L], scr[:, L : L + QW],
                    start=True, stop=True, skip_group_check=True,
                )

            # --- loads (SP + GpSimd rings) ---
            # slab 0 in geometrically growing pieces (W+u0, u1, u2-3, u4-7,
            # u8-15) so the first MM fires as early as possible.
            UB = 513  # bytes per unit
            cuts = [0, WBYTES + UB, WBYTES + 2 * UB, WBYTES + 4 * UB,
                    WBYTES + 8 * UB, WBYTES + 12 * UB, WBYTES + 16 * UB]
            for k in range(6):
                ring = nc.sync if k % 2 == 0 else nc.gpsimd
                ring.dma_start(
                    xts[0][:, cuts[k] : cuts[k + 1]],
                    x0[:, cuts[k] : cuts[k + 1]],
                )
            half = SROW // 2
            for q in range(1, NSLAB):
                nc.sync.dma_start(
                    xts[q][:, 0:half], xs_r[q - 1][:, 0:half]
                )
                nc.gpsimd.dma_start(
                    xts[q][:, half:], xs_r[q - 1][:, half:]
                )

            t1s = xts[0][:, 0:L]
            t0s = xts[0][:, L : 2 * L]

            # --- compute, slab-major; two plain fp8 matmuls per chunk ---
            for q in range(NSLAB):
                xt = xts[q]
                yo = yos[q]
                base = ORIG if q == 0 else 0
                for u in range(NU):
                    ps = ps_pool.tile([L, QW], f32, tag="mm")
                    c0 = base + 513 * u + 1  # byte of X[512q] for unit u
                    nc.tensor.matmul(
                        ps[:], t0s, xt[:, c0 : c0 + QW],
                        start=True, stop=False,
                    )
                    nc.tensor.matmul(
                        ps[:], t1s, xt[:, c0 - 1 : c0 + QW - 1],
                        start=False, stop=True,
                    )
                    dst = yo[:, QW * u : QW * (u + 1)]
                    if q == NSLAB - 1 and u >= 14:
                        # tail: halve the last evacs across both engines
                        nc.vector.tensor_copy(dst[:, 0 : QW // 2],
                                              ps[:, 0 : QW // 2])
                        nc.scalar.copy(dst[:, QW // 2 :], ps[:, QW // 2 :])
                    elif u % 2 == 0:
                        nc.vector.tensor_copy(dst, ps[:])
                    else:
                        nc.scalar.copy(dst, ps[:])

                # store unit-group pieces as they finish; finer on the last
                # slab to shorten the tail, coarser early (fewer semaphores
                # shortens the end-of-program wait drain).
                nparts = 4 if q < NSLAB - 1 else 16
                sq = NU * QW // nparts
                for k in range(nparts):
                    ring = nc.sync if k % 2 == 0 else nc.gpsimd
                    ring.dma_start(
                        y_r[q][:, k * sq : (k + 1) * sq],
                        yo[:, k * sq : (k + 1) * sq],
                    )

    return nc


def _strip_redundant_waits(bir_bytes: bytes) -> bytes:
    """PE Matmult/Ldweights lower to TPB instructions with a single
    semaphore-wait slot, but Tile's slot-release deps put 2 waits (old-writer
    PE completion + old-reader DVE completion) on the first toucher of every
    reused PSUM slot.  The PE wait is transitively implied: the DVE evac copy
    whose completion the instruction also waits on had itself waited on those
    PE completions.  Prove the implication with a completion-guarantee
    dataflow (rules: an instruction completes only after its waits hold; TPB
    engine queues are in-order FIFO; a semaphore's v-th update implies its
    earlier updates) and drop provably-redundant waits; raise if a >1-wait
    matmul can't be reduced."""
    import json

    bir = json.loads(bir_bytes)
    insts = []
    containers = []  # (list, index) for each inst, for NoOp insertion

    def walk(block):
        lst = block.get("instructions", [])
        for idx, i in enumerate(lst):
            insts.append(i)
            containers.append((lst, idx))
        for sub in block.get("blocks", []):
            walk(sub)

    for b in bir["functions"][0]["blocks"]:
        walk(b)

    # Per-sem update timeline: list of (cumulative_value, inst_idx).
    timelines = {}
    for k, i in enumerate(insts):
        for u in i.get("sync_info", {}).get("on_update", []) or []:
            if u.get("sync_type") != "semaphore":
                continue
            tl = timelines.setdefault(u["ant_name"], [])
            prev = tl[-1][0] if tl else 0
            tl.append((prev + int(u.get("update_value", 1)), k))

    def producer(sem, val):
        """Index of the instruction whose update first brings sem >= val."""
        tl = timelines.get(sem)
        if not tl:
            return None
        import bisect
        pos = bisect.bisect_left(tl, (val, -1))
        if pos == len(tl):
            return None
        return tl[pos][1]

    IN_ORDER_ENGINES = {"PE", "DVE", "Activation", "Pool", "SP"}
    NOT_IN_ORDER_OPCODES = {"DMACopy", "DmaTransposeAnt"}  # complete out-of-band

    # guarantees[k]: sem -> max value known to hold when inst k completes.
    guarantees = [dict() for _ in insts]
    prev_by_engine = {}
    preds = []  # per-inst: same-engine predecessor (in-order engines only)
    for k, i in enumerate(insts):
        eng = i.get("engine")
        in_order = eng in IN_ORDER_ENGINES and i.get("opcode") not in NOT_IN_ORDER_OPCODES
        pred = prev_by_engine.get(eng) if in_order else None
        preds.append(pred)
        if in_order:
            prev_by_engine[eng] = k

    def merge(dst, src):
        changed = False
        for s, v in src.items():
            if dst.get(s, 0) < v:
                dst[s] = v
                changed = True
        return changed

    for _pass in range(3):
        changed = False
        for k, i in enumerate(insts):
            g = guarantees[k]
            si = i.get("sync_info", {})
            for w in si.get("on_wait", []) or []:
                if w.get("sync_type") != "semaphore":
                    continue
                v = int(w["wait_value"])
                if g.get(w["ant_name"], 0) < v:
                    g[w["ant_name"]] = v
                    changed = True
                p = producer(w["ant_name"], v)
                if p is not None:
                    changed |= merge(g, guarantees[p])
            if preds[k] is not None:
                changed |= merge(g, guarantees[preds[k]])
        # Own updates fire at completion; same-sem update chains are FIFO
        # (engine queue or DMA queue), so the v-th updater inherits the
        # (v-1)-th updater's guarantees.
        for sem, tl in timelines.items():
            prev_idx = None
            for cum, k in tl:
                if guarantees[k].get(sem, 0) < cum:
                    guarantees[k][sem] = cum
                    changed = True
                if prev_idx is not None:
                    changed |= merge(guarantees[k], guarantees[prev_idx])
                prev_idx = k
        if not changed:
            break

    STRIP_OPCODES = {
        "Matmult", "Ldweights", "TensorCopy", "Memset", "DMACopy",
        "DmaTransposeAnt",
        "Activation", "TensorScalarAffineSelect", "TensorTensor",
        "TensorScalarPtr", "TensorReduce", "Drain", "NoOp",
    }
    stripped = 0
    inserts = []  # (list, index, [noop dicts])
    for k, i in enumerate(insts):
        if i.get("opcode") not in STRIP_OPCODES:
            continue
        si = i.get("sync_info", {})
        waits = si.get("on_wait", []) or []
        if len(waits) <= 1:
            continue
        # Drop every wait implied by another (not-yet-dropped) wait's
        # producer guarantee.
        kept = list(waits)
        changed = True
        while changed:
            changed = False
            for w in list(kept):
                if len(kept) == 1:
                    break
                for w2 in kept:
                    if w2 is w:
                        continue
                    p = producer(w2["ant_name"], int(w2["wait_value"]))
                    if p is not None and guarantees[p].get(w["ant_name"], 0) >= int(
                        w["wait_value"]
                    ):
                        kept.remove(w)
                        changed = True
                        break
        stripped += len(waits) - len(kept)
        si["on_wait"] = [kept[-1]]
        if len(kept) > 1:
            # Split remaining waits onto single-wait NoOps ahead of the
            # instruction on the same engine queue.
            lst, idx = containers[k]
            noops = [
                {
                    "debug": i.get("debug", 0),
                    "engine": i.get("engine"),
                    "ins": [],
                    "name": f"{i['name']}-w{j}",
                    "opcode": "NoOp",
                    "outs": [],
                    "sync_info": {"on_wait": [w], "on_update": []},
                }
                for j, w in enumerate(kept[:-1])
            ]
            inserts.append((lst, idx, noops))

    # Apply insertions (descending index per list keeps positions valid).
    from collections import defaultdict
    by_list = defaultdict(list)
    for lst, idx, noops in inserts:
        by_list[id(lst)].append((lst, idx, noops))
    for entries in by_list.values():
        for lst, idx, noops in sorted(entries, key=lambda e: -e[1]):
            lst[idx:idx] = noops

    out = json.dumps(bir).encode()
    return out


def audit_waits(bir_bytes):
    """Flag instructions with more than the single hardware wait slot."""
    import json

    bir = json.loads(bir_bytes)
    checked = {
        "Matmult", "Ldweights", "TensorCopy", "Memset", "DMACopy",
        "DmaTransposeAnt",
        "Activation", "TensorScalarAffineSelect", "TensorTensor",
        "TensorScalarPtr", "TensorReduce",
    }
    bad = []
    def walk(block):
        for i in block.get("instructions", []):
            if i.get("opcode") not in checked:
                continue
            w = i.get("sync_info", {}).get("on_wait", [])
            if len(w) > 1:
                bad.append((i["name"], i.get("opcode"), i.get("engine"),
                            [(x["ant_name"], x["wait_value"]) for x in w]))
        for sub in block.get("blocks", []):
            walk(sub)
    for b in bir["functions"][0]["blocks"]:
        walk(b)
    return bad


def _get_nc():
    if "nc" not in _NC_CACHE:
        nc = _build_nc()
        patched = _strip_redundant_waits(type(nc).to_json_bytes(nc))
        bad = audit_waits(patched)
        if bad:
            raise RuntimeError(f"multi-wait instructions remain: {bad[:5]}")
        nc.to_json_bytes = lambda: patched
        _NC_CACHE["nc"] = nc
    return _NC_CACHE["nc"]


def _host_prep(x64_f32: np.ndarray, t0t: np.ndarray, t1t: np.ndarray):
    """fp32 [64, T] -> per-core in_maps with fp8 interleaved slab layout."""
    import ml_dtypes

    f8 = ml_dtypes.float8_e4m3fn
    # block-major X'[s, j, B] then fp8
    xbm = np.ascontiguousarray(
        x64_f32.reshape(SIGS, NBLK, L).transpose(0, 2, 1)
    ).astype(f8)
    w8 = np.concatenate([t1t, t0t], axis=1).astype(f8)  # [128, 256] (T1|T0)

    in_maps = []
    for c in range(N_CORES):
        sig = xbm[SPC * c : SPC * (c + 1)]             # [8, 128, 4096]
        units = sig.reshape(SPC, L, 2, UW).transpose(0, 2, 1, 3).reshape(
            NU, L, UW
        )                                               # u = 2s+h
        # halo col per (slab, unit): col 512q-1 of the unit (zeros for
        # q==0 & first-half units; second-half q==0 halo = col 2047 of
        # the signal's first half = units[u-1][:, -1]).
        slabs = np.zeros((NSLAB, L, SROW), dtype=f8)
        for q in range(NSLAB):
            halo = np.zeros((NU, L), dtype=f8)
            if q == 0:
                halo[1::2] = units[0::2, :, UW - 1]
            else:
                halo[:] = units[:, :, QW * q - 1]
            block = units[:, :, QW * q : QW * (q + 1)]  # [16, 128, 512]
            unit_rows = np.concatenate(
                [halo[:, :, None], block], axis=2
            )  # [16, 128, 513]: unit-major, halo col first
            slabs[q] = np.ascontiguousarray(
                unit_rows.transpose(1, 0, 2)
            ).reshape(L, SROW)
        x0 = np.ascontiguousarray(
            np.concatenate([w8, slabs[0]], axis=1)
        )
        in_maps.append({"x0": x0, "xs": np.ascontiguousarray(slabs[1:])})
    return in_maps


def _host_finish(results, x_f32: np.ndarray):
    """Per-core y slabs -> d [64, T] fp32; return y = x + d."""
    d64 = np.empty((SIGS, T_FULL), dtype=np.float32)
    for c in range(N_CORES):
        ys = np.asarray(results[c]["y"]).astype(np.float32)  # [4, 128, 8192]
        # ys[q][p][512u+j] = unit u col 512q+j
        units = ys.reshape(NSLAB, L, NU, QW).transpose(2, 1, 0, 3).reshape(
            NU, L, UW
        )
        sig = units.reshape(SPC, 2, L, UW).transpose(0, 2, 1, 3).reshape(
            SPC, L, NBLK
        )
        d64[SPC * c : SPC * (c + 1)] = (
            sig.transpose(0, 2, 1).reshape(SPC, T_FULL)
        )
    return x_f32 + d64


def run_spmd(x64: np.ndarray, t0t: np.ndarray, t1t: np.ndarray, trace: bool = False):
    """x64: [64, T] float32 -> [64, T] float32 (plus BassKernelResults)."""
    from concourse.bass_utils import run_bass_kernel_spmd

    nc = _get_nc()
    in_maps = _host_prep(x64, t0t, t1t)
    res = run_bass_kernel_spmd(
        nc, in_maps, core_ids=list(range(N_CORES)), trace=trace
    )
    out = _host_finish(res.results, x64)
    return out, res


def kernel(x, center_freq, q, gain, t=0, **_unused):
    x = np.ascontiguousarray(np.asarray(x), dtype=np.float32)
    assert x.shape == (B_FULL, C_FULL, T_FULL), x.shape
    cf = float(np.asarray(center_freq).reshape(-1)[0])
    qv = float(np.asarray(q).reshape(-1)[0])
    gv = float(np.asarray(gain).reshape(-1)[0])

    h = _impulse_response(cf, qv, gv)
    h[0] -= 1.0  # residual filter: d = y - x
    t0t, t1t = _toeplitz_mats(h)

    x64 = x.reshape(SIGS, T_FULL)
    out, _ = run_spmd(x64, t0t, t1t, trace=False)
    return out.reshape(B_FULL, C_FULL, T_FULL).astype(np.float32)


# revision 7
# speedup vs baseline: 1.0275x; 1.0060x over previous
"""Biquad peaking-EQ IIR on 8 Trainium2 NeuronCores — fp8 residual edition.

Math: the reference filter y = H(x) is a 2nd-order IIR whose impulse response
decays below 1e-10 after 256 taps; we compute the *residual* d = y - x as a
truncated-FIR convolution (taps g = h - delta), quantize everything to
fp8 e4m3, and reconstruct y = x + d on the host (which already holds x in
fp32).  End-to-end rel-L2 error ~6e-3 (numpy-validated), under the 2e-2 gate.

Performance design (59.7us bf16 ancestor -> ~45.5us measured):
 1. fp8 e4m3 input AND output halve DMA bytes to 8.4 MB/core -> ~24us of
    HBM flow at the ~350 B/ns per-core aggregate rate.
 2. Two plain fp8 matmuls per 512-block chunk (T0 x X[c] + T1 x X[c-1],
    fp32 PSUM accumulation); back-to-back MMs issue every ~216 ns (full
    2.4 GHz), so the 128-MM PE stream is ~27.6us and is the critical
    stream.  (fp8 DoubleRow would halve MM count but requires the moving
    k-pair stride to be a multiple of 16 bytes: overlapping stride-1
    views crash the PE, a 16-byte-interleaved layout runs the strided
    fetch 2.8x slow, and building an on-chip shifted duplicate costs more
    than it saves - SBUF->SBUF DMA measured ~163-260 B/ns with multi-us
    completion lag, and engine copies are parity-blocked on the odd
    1-byte shift.  All measured; plain wins.)
 3. Work streams in 4 column-slabs (512 block cols x 16 half-signal
    units, ~1.05 MB).  Each slab tile carries its own leading halo col
    per unit (host-duplicated), so slabs are fully independent; loads and
    stores split across the SP and GpSimd HWDGE rings (2 queues x ~200
    B/ns covers the ~350 B/ns core cap); compute is slab-major so output
    retires evenly; stores go out in unit-group quarters (eighths on the
    last slab) right behind the evacuations to minimize the tail.
 4. PSUM chunk tiles [128,512] f32 (7 bufs); evacuation (fp32->fp8 cast,
    ~687ns/chunk) alternates DVE/ACT per unit (~22us each, hidden under
    the PE stream).  The Toeplitz pair rides the front of slab 0's first
    load piece; slab 0 loads in small pieces so the first MM fires ~10.5us.
 5. PE pstate warmup: 5 dummy matmuls on a zeroed scratch tile keep the
    PE busy from ~8.3us so real MMs reach the full clock quickly (the
    PE downclocks unless continuously busy ~3us).

Scheduling note: TPB instructions have a single semaphore-wait slot; Tile's
slot-release deps routinely emit 2+ waits.  _strip_redundant_waits
post-processes the scheduled BIR (see its docstring).
"""

import math

import numpy as np

SAMPLE_RATE = 44100.0

# Problem geometry (hardcoded per harness contract).
B_FULL, C_FULL, T_FULL = 32, 2, 524288
N_CORES = 8
SIGS = B_FULL * C_FULL          # 64 signals
SPC = SIGS // N_CORES           # 8 signals per core
L = 128                         # block size == PE array dim
NBLK = T_FULL // L              # 4096 block cols per signal
NU = 16                         # interleaved half-signal units per core
UW = NBLK // 2                  # 2048 block cols per unit
QW = 512                        # block cols per chunk (= PSUM bank, fp32)
NSLAB = UW // QW                # 4 slabs
SROW = NU * (1 + QW)            # slab row bytes: 16 units x (halo col + 512)
WBYTES = 2 * L                  # Toeplitz pair rides the front of slab 0


def _filter_coeffs(center_freq: float, q: float, gain: float):
    """torchaudio equalizer_biquad coefficients, normalized by a0 (float64)."""
    g = min(max(gain, 0.1), 10.0)
    w0 = 2.0 * math.pi * center_freq / SAMPLE_RATE
    A = math.exp(g / 40.0 * math.log(10.0))
    alpha = math.sin(w0) / (2.0 * q)
    b0 = 1.0 + alpha * A
    b1 = -2.0 * math.cos(w0)
    b2 = 1.0 - alpha * A
    a0 = 1.0 + alpha / A
    a1 = b1
    a2 = 1.0 - alpha / A
    return b0 / a0, b1 / a0, b2 / a0, a1 / a0, a2 / a0


def _impulse_response(center_freq: float, q: float, gain: float, n: int = 256):
    b0, b1, b2, a1, a2 = _filter_coeffs(center_freq, q, gain)
    h = np.zeros(n, dtype=np.float64)
    x1 = x2 = y1 = y2 = 0.0
    for i in range(n):
        xn = 1.0 if i == 0 else 0.0
        yn = b0 * xn + b1 * x1 + b2 * x2 - a1 * y1 - a2 * y2
        x2, x1 = x1, xn
        y2, y1 = y1, yn
        h[i] = yn
    return h


def _toeplitz_mats(h: np.ndarray):
    """T0T[j,g] = h[g-j] (g>=j else 0); T1T[j,g] = h[128+g-j].  Stationary
    matmul operands, contracting over partition j."""
    j = np.arange(L)[:, None]
    g = np.arange(L)[None, :]
    d0 = g - j
    t0t = np.where(d0 >= 0, h[np.clip(d0, 0, len(h) - 1)], 0.0)
    d1 = 128 + g - j
    t1t = h[np.clip(d1, 0, len(h) - 1)]
    return t0t.astype(np.float32), t1t.astype(np.float32)


_NC_CACHE = {}


def _build_nc():
    """Per-core Bass program (same NEFF on all cores).

    dram x0: [128, WBYTES + SROW]  = [T1|T0 | slab-0 bytes]
    dram xs: [NSLAB-1, 128, SROW]    (slabs 1..3, each with own halo col)
    dram y:  [NSLAB, 128, NU*QW]     (slab-major, units side by side)

    Slab tile layout [128, SROW] (+WBYTES Toeplitz prefix on slab 0):
      unit u's data at byte 513u; col 512q+c'-1 at +c' (c'=0 = halo col).

    Dispatch plan: all loads first on the SP+GpSimd rings (slab 0 split in
    small leading pieces so the MM stream starts early), then slab-major
    compute with stores split on SP+GpSimd.
    """
    import concourse.bass as bass
    import concourse.mybir as mybir
    import concourse.tile as tile
    from concourse.ap import AP

    f8 = mybir.dt.float8e4
    f32 = mybir.dt.float32
    nc = bass.Bass("TRN2")

    x0 = nc.dram_tensor("x0", [L, WBYTES + SROW], f8, kind="ExternalInput")
    xs = nc.dram_tensor("xs", [NSLAB - 1, L, SROW], f8, kind="ExternalInput")
    y = nc.dram_tensor("y", [NSLAB, L, NU * QW], f8, kind="ExternalOutput")
    xs_r = xs[:]
    y_r = y[:]

    QTR = SROW // 4   # 2052 B = 4 units
    ORIG = WBYTES     # slab data offset within the slab-0 tile
    with tile.TileContext(nc) as tc:
        with (
            tc.tile_pool(name="xt", bufs=1) as xt_pool,
            tc.tile_pool(name="yo", bufs=1) as yo_pool,
            tc.tile_pool(name="ps", bufs=7, space="PSUM") as ps_pool,
            tc.tile_pool(name="psw", bufs=1, space="PSUM") as psw_pool,
        ):
            xts = [
                xt_pool.tile(
                    [L, SROW + (WBYTES if q == 0 else 0)], f8, name=f"xt{q}"
                )
                for q in range(NSLAB)
            ]
            yos = [
                yo_pool.tile([L, NU * QW], f8, name=f"yo{q}")
                for q in range(NSLAB)
            ]

            # PE pstate warmup: dummy matmuls on scratch keep the PE busy
            # through the load window so real MMs start at full clock.
            scr = xt_pool.tile([L, L + QW], f8, name="scratch")
            nc.gpsimd.memset(scr[:, 0 : (L + QW) // 2], 0)
            nc.vector.memset(scr[:, (L + QW) // 2 :], 0)
            ps_w = psw_pool.tile([L, QW], f32, name="ps_warm")
            for _ in range(4):
                nc.tensor.matmul(
                    ps_w[:], scr[:, 0:L], scr[:, L : L + QW],
                    start=True, stop=True, skip_group_check=True,
                )

            # --- loads (SP + GpSimd rings) ---
            # slab 0 in geometrically growing pieces (W+u0, u1, u2-3, u4-7,
            # u8-15) so the first MM fires as early as possible.
            UB = 513  # bytes per unit
            cuts = [0, WBYTES + UB, WBYTES + 2 * UB, WBYTES + 3 * UB,
                    WBYTES + 4 * UB, WBYTES + 6 * UB, WBYTES + 8 * UB,
                    WBYTES + 12 * UB, WBYTES + 16 * UB]
            for k in range(8):
                ring = nc.sync if k % 2 == 0 else nc.gpsimd
                ring.dma_start(
                    xts[0][:, cuts[k] : cuts[k + 1]],
                    x0[:, cuts[k] : cuts[k + 1]],
                )
            half = SROW // 2
            for q in range(1, NSLAB):
                nc.sync.dma_start(
                    xts[q][:, 0:half], xs_r[q - 1][:, 0:half]
                )
                nc.gpsimd.dma_start(
                    xts[q][:, half:], xs_r[q - 1][:, half:]
                )

            t1s = xts[0][:, 0:L]
            t0s = xts[0][:, L : 2 * L]

            # --- compute, slab-major; two plain fp8 matmuls per chunk ---
            for q in range(NSLAB):
                xt = xts[q]
                yo = yos[q]
                base = ORIG if q == 0 else 0
                for u in range(NU):
                    ps = ps_pool.tile([L, QW], f32, tag="mm")
                    c0 = base + 513 * u + 1  # byte of X[512q] for unit u
                    nc.tensor.matmul(
                        ps[:], t0s, xt[:, c0 : c0 + QW],
                        start=True, stop=False,
                    )
                    nc.tensor.matmul(
                        ps[:], t1s, xt[:, c0 - 1 : c0 + QW - 1],
                        start=False, stop=True,
                    )
                    dst = yo[:, QW * u : QW * (u + 1)]
                    if q == NSLAB - 1 and u >= 14:
                        # tail: halve the last evacs across both engines
                        nc.vector.tensor_copy(dst[:, 0 : QW // 2],
                                              ps[:, 0 : QW // 2])
                        nc.scalar.copy(dst[:, QW // 2 :], ps[:, QW // 2 :])
                    elif u % 2 == 0:
                        nc.vector.tensor_copy(dst, ps[:])
                    else:
                        nc.scalar.copy(dst, ps[:])

                # store unit-group pieces as they finish; finer on the last
                # slab to shorten the tail, coarser early (fewer semaphores
                # shortens the end-of-program wait drain).
                nparts = 4 if q < NSLAB - 1 else 16
                sq = NU * QW // nparts
                for k in range(nparts):
                    ring = nc.sync if k % 2 == 0 else nc.gpsimd
                    ring.dma_start(
                        y_r[q][:, k * sq : (k + 1) * sq],
                        yo[:, k * sq : (k + 1) * sq],
                    )

    return nc


def _strip_redundant_waits(bir_bytes: bytes) -> bytes:
    """PE Matmult/Ldweights lower to TPB instructions with a single
    semaphore-wait slot, but Tile's slot-release deps put 2 waits (old-writer
    PE completion + old-reader DVE completion) on the first toucher of every
    reused PSUM slot.  The PE wait is transitively implied: the DVE evac copy
    whose completion the instruction also waits on had itself waited on those
    PE completions.  Prove the implication with a completion-guarantee
    dataflow (rules: an instruction completes only after its waits hold; TPB
    engine queues are in-order FIFO; a semaphore's v-th update implies its
    earlier updates) and drop provably-redundant waits; raise if a >1-wait
    matmul can't be reduced."""
    import json

    bir = json.loads(bir_bytes)
    insts = []
    containers = []  # (list, index) for each inst, for NoOp insertion

    def walk(block):
        lst = block.get("instructions", [])
        for idx, i in enumerate(lst):
            insts.append(i)
            containers.append((lst, idx))
        for sub in block.get("blocks", []):
            walk(sub)

    for b in bir["functions"][0]["blocks"]:
        walk(b)

    # Per-sem update timeline: list of (cumulative_value, inst_idx).
    timelines = {}
    for k, i in enumerate(insts):
        for u in i.get("sync_info", {}).get("on_update", []) or []:
            if u.get("sync_type") != "semaphore":
                continue
            tl = timelines.setdefault(u["ant_name"], [])
            prev = tl[-1][0] if tl else 0
            tl.append((prev + int(u.get("update_value", 1)), k))

    def producer(sem, val):
        """Index of the instruction whose update first brings sem >= val."""
        tl = timelines.get(sem)
        if not tl:
            return None
        import bisect
        pos = bisect.bisect_left(tl, (val, -1))
        if pos == len(tl):
            return None
        return tl[pos][1]

    IN_ORDER_ENGINES = {"PE", "DVE", "Activation", "Pool", "SP"}
    NOT_IN_ORDER_OPCODES = {"DMACopy", "DmaTransposeAnt"}  # complete out-of-band

    # guarantees[k]: sem -> max value known to hold when inst k completes.
    guarantees = [dict() for _ in insts]
    prev_by_engine = {}
    preds = []  # per-inst: same-engine predecessor (in-order engines only)
    for k, i in enumerate(insts):
        eng = i.get("engine")
        in_order = eng in IN_ORDER_ENGINES and i.get("opcode") not in NOT_IN_ORDER_OPCODES
        pred = prev_by_engine.get(eng) if in_order else None
        preds.append(pred)
        if in_order:
            prev_by_engine[eng] = k

    def merge(dst, src):
        changed = False
        for s, v in src.items():
            if dst.get(s, 0) < v:
                dst[s] = v
                changed = True
        return changed

    for _pass in range(3):
        changed = False
        for k, i in enumerate(insts):
            g = guarantees[k]
            si = i.get("sync_info", {})
            for w in si.get("on_wait", []) or []:
                if w.get("sync_type") != "semaphore":
                    continue
                v = int(w["wait_value"])
                if g.get(w["ant_name"], 0) < v:
                    g[w["ant_name"]] = v
                    changed = True
                p = producer(w["ant_name"], v)
                if p is not None:
                    changed |= merge(g, guarantees[p])
            if preds[k] is not None:
                changed |= merge(g, guarantees[preds[k]])
        # Own updates fire at completion; same-sem update chains are FIFO
        # (engine queue or DMA queue), so the v-th updater inherits the
        # (v-1)-th updater's guarantees.
        for sem, tl in timelines.items():
            prev_idx = None
            for cum, k in tl:
                if guarantees[k].get(sem, 0) < cum:
                    guarantees[k][sem] = cum
                    changed = True
                if prev_idx is not None:
                    changed |= merge(guarantees[k], guarantees[prev_idx])
                prev_idx = k
        if not changed:
            break

    STRIP_OPCODES = {
        "Matmult", "Ldweights", "TensorCopy", "Memset", "DMACopy",
        "DmaTransposeAnt",
        "Activation", "TensorScalarAffineSelect", "TensorTensor",
        "TensorScalarPtr", "TensorReduce", "Drain", "NoOp",
    }
    stripped = 0
    inserts = []  # (list, index, [noop dicts])
    for k, i in enumerate(insts):
        if i.get("opcode") not in STRIP_OPCODES:
            continue
        si = i.get("sync_info", {})
        waits = si.get("on_wait", []) or []
        if len(waits) <= 1:
            continue
        # Drop every wait implied by another (not-yet-dropped) wait's
        # producer guarantee.
        kept = list(waits)
        changed = True
        while changed:
            changed = False
            for w in list(kept):
                if len(kept) == 1:
                    break
                for w2 in kept:
                    if w2 is w:
                        continue
                    p = producer(w2["ant_name"], int(w2["wait_value"]))
                    if p is not None and guarantees[p].get(w["ant_name"], 0) >= int(
                        w["wait_value"]
                    ):
                        kept.remove(w)
                        changed = True
                        break
        stripped += len(waits) - len(kept)
        si["on_wait"] = [kept[-1]]
        if len(kept) > 1:
            # Split remaining waits onto single-wait NoOps ahead of the
            # instruction on the same engine queue.
            lst, idx = containers[k]
            noops = [
                {
                    "debug": i.get("debug", 0),
                    "engine": i.get("engine"),
                    "ins": [],
                    "name": f"{i['name']}-w{j}",
                    "opcode": "NoOp",
                    "outs": [],
                    "sync_info": {"on_wait": [w], "on_update": []},
                }
                for j, w in enumerate(kept[:-1])
            ]
            inserts.append((lst, idx, noops))

    # Apply insertions (descending index per list keeps positions valid).
    from collections import defaultdict
    by_list = defaultdict(list)
    for lst, idx, noops in inserts:
        by_list[id(lst)].append((lst, idx, noops))
    for entries in by_list.values():
        for lst, idx, noops in sorted(entries, key=lambda e: -e[1]):
            lst[idx:idx] = noops

    out = json.dumps(bir).encode()
    return out


def audit_waits(bir_bytes):
    """Flag instructions with more than the single hardware wait slot."""
    import json

    bir = json.loads(bir_bytes)
    checked = {
        "Matmult", "Ldweights", "TensorCopy", "Memset", "DMACopy",
        "DmaTransposeAnt",
        "Activation", "TensorScalarAffineSelect", "TensorTensor",
        "TensorScalarPtr", "TensorReduce",
    }
    bad = []
    def walk(block):
        for i in block.get("instructions", []):
            if i.get("opcode") not in checked:
                continue
            w = i.get("sync_info", {}).get("on_wait", [])
            if len(w) > 1:
                bad.append((i["name"], i.get("opcode"), i.get("engine"),
                            [(x["ant_name"], x["wait_value"]) for x in w]))
        for sub in block.get("blocks", []):
            walk(sub)
    for b in bir["functions"][0]["blocks"]:
        walk(b)
    return bad


def _get_nc():
    if "nc" not in _NC_CACHE:
        nc = _build_nc()
        patched = _strip_redundant_waits(type(nc).to_json_bytes(nc))
        bad = audit_waits(patched)
        if bad:
            raise RuntimeError(f"multi-wait instructions remain: {bad[:5]}")
        nc.to_json_bytes = lambda: patched
        _NC_CACHE["nc"] = nc
    return _NC_CACHE["nc"]


def _host_prep(x64_f32: np.ndarray, t0t: np.ndarray, t1t: np.ndarray):
    """fp32 [64, T] -> per-core in_maps with fp8 interleaved slab layout."""
    import ml_dtypes

    f8 = ml_dtypes.float8_e4m3fn
    # block-major X'[s, j, B] then fp8
    xbm = np.ascontiguousarray(
        x64_f32.reshape(SIGS, NBLK, L).transpose(0, 2, 1)
    ).astype(f8)
    w8 = np.concatenate([t1t, t0t], axis=1).astype(f8)  # [128, 256] (T1|T0)

    in_maps = []
    for c in range(N_CORES):
        sig = xbm[SPC * c : SPC * (c + 1)]             # [8, 128, 4096]
        units = sig.reshape(SPC, L, 2, UW).transpose(0, 2, 1, 3).reshape(
            NU, L, UW
        )                                               # u = 2s+h
        # halo col per (slab, unit): col 512q-1 of the unit (zeros for
        # q==0 & first-half units; second-half q==0 halo = col 2047 of
        # the signal's first half = units[u-1][:, -1]).
        slabs = np.zeros((NSLAB, L, SROW), dtype=f8)
        for q in range(NSLAB):
            halo = np.zeros((NU, L), dtype=f8)
            if q == 0:
                halo[1::2] = units[0::2, :, UW - 1]
            else:
                halo[:] = units[:, :, QW * q - 1]
            block = units[:, :, QW * q : QW * (q + 1)]  # [16, 128, 512]
            unit_rows = np.concatenate(
                [halo[:, :, None], block], axis=2
            )  # [16, 128, 513]: unit-major, halo col first
            slabs[q] = np.ascontiguousarray(
                unit_rows.transpose(1, 0, 2)
            ).reshape(L, SROW)
        x0 = np.ascontiguousarray(
            np.concatenate([w8, slabs[0]], axis=1)
        )
        in_maps.append({"x0": x0, "xs": np.ascontiguousarray(slabs[1:])})
    return in_maps


def _host_finish(results, x_f32: np.ndarray):
    """Per-core y slabs -> d [64, T] fp32; return y = x + d."""
    d64 = np.empty((SIGS, T_FULL), dtype=np.float32)
    for c in range(N_CORES):
        ys = np.asarray(results[c]["y"]).astype(np.float32)  # [4, 128, 8192]
        # ys[q][p][512u+j] = unit u col 512q+j
        units = ys.reshape(NSLAB, L, NU, QW).transpose(2, 1, 0, 3).reshape(
            NU, L, UW
        )
        sig = units.reshape(SPC, 2, L, UW).transpose(0, 2, 1, 3).reshape(
            SPC, L, NBLK
        )
        d64[SPC * c : SPC * (c + 1)] = (
            sig.transpose(0, 2, 1).reshape(SPC, T_FULL)
        )
    return x_f32 + d64


def run_spmd(x64: np.ndarray, t0t: np.ndarray, t1t: np.ndarray, trace: bool = False):
    """x64: [64, T] float32 -> [64, T] float32 (plus BassKernelResults)."""
    from concourse.bass_utils import run_bass_kernel_spmd

    nc = _get_nc()
    in_maps = _host_prep(x64, t0t, t1t)
    res = run_bass_kernel_spmd(
        nc, in_maps, core_ids=list(range(N_CORES)), trace=trace
    )
    out = _host_finish(res.results, x64)
    return out, res


def kernel(x, center_freq, q, gain, t=0, **_unused):
    x = np.ascontiguousarray(np.asarray(x), dtype=np.float32)
    assert x.shape == (B_FULL, C_FULL, T_FULL), x.shape
    cf = float(np.asarray(center_freq).reshape(-1)[0])
    qv = float(np.asarray(q).reshape(-1)[0])
    gv = float(np.asarray(gain).reshape(-1)[0])

    h = _impulse_response(cf, qv, gv)
    h[0] -= 1.0  # residual filter: d = y - x
    t0t, t1t = _toeplitz_mats(h)

    x64 = x.reshape(SIGS, T_FULL)
    out, _ = run_spmd(x64, t0t, t1t, trace=False)
    return out.reshape(B_FULL, C_FULL, T_FULL).astype(np.float32)
